# revision 38
# baseline (speedup 1.0000x reference)
"""Trainium2 Bass kernel: Bahdanau local-p attention (B=32, S=2048, H=1024).

Sharding: data-parallel over batch. Each of the 8 cores processes B/8 = 4
batches end-to-end (weights replicated); no collectives.

Key insight vs the naive pipeline: SWDGE cast-DMAs are element-rate
limited (~20 G elem/s -> ~90us per 2.1M-element cast), so no
dtype-converting DMA is ever issued. Per-core dataflow:
  1. x[b] fp32 loaded natural in [256-row] chunks (plain DMA, full
     rate), cast fp32->fp8e4 on DVE/GpSimd, stored fp8 to DRAM (plain),
     then u16-viewed xbar transposes -> SBUF xt [128, 4j, 2048] u16,
     pair-interleaved: partition p of block j holds bytes
     (h=256j+2p, h=256j+2p+1) along s.
  2. Main matmul in fp8 DoubleRow (2x PE rate): lhsT = W_a [128,2,128]
     in matching (j p two) order, rhs = interleaved fp8 view
     [128, 2(stride 1B), 512(stride 2B)]; K=256 per instruction.
  3. tanh(WH^T + U_a h_t) on ACT with per-partition bias, fp8 out.
  4. score via DoubleRow v_a matmuls; s-block q accumulates into row q
     of one [16,512] PSUM bank (v_a placed in weight column q).
  5. softmax (no max-sub; scores are O(1)) * gaussian on [4,512] rows;
     weight row scaled by 256 for fp8 range.
  6. selector-matmul replicates the weight row to 128 partitions;
     context via 8 strided fp8 DVE multiply-accumulates (one per (j,i)).
  7. final tanh([ctx, h_t] @ W_att) in fp16; W_att ctx rows loaded in
     the same (j p two) permuted order so the context needs no shuffle.
All weights are plain fp32 DMA loads + engine casts (fp16/fp8).
"""

import math
from contextlib import ExitStack

import numpy as np

B, S, H, SIZE = 32, 2048, 1024, 1024
N_CORES = 8
BPC = B // N_CORES
P = 128
NB = 512

_compiled = None


def _build(bpc=BPC, s=S, h=H, size=SIZE, debug=False):
    import concourse.bacc as bacc
    import concourse.mybir as mybir
    import concourse.tile as tile

    F32 = mybir.dt.float32
    F16 = mybir.dt.float16
    BF16 = mybir.dt.bfloat16
    F8 = mybir.dt.float8e4
    U16 = mybir.dt.uint16
    AF = mybir.ActivationFunctionType
    ALU = mybir.AluOpType
    AX = mybir.AxisListType
    DR = mybir.MatmulPerfMode.DoubleRow

    KT = h // P            # 8 h-tiles of 128
    SQ = s // NB           # 4 s-blocks of 512
    KT2 = 2 * h // P       # 16 k-tiles for the final projection
    NO = size // NB        # 2 output blocks of 512
    denom = 2.0 * ((s // 2) / 2.0) ** 2
    inv_sq_denom = 1.0 / math.sqrt(denom)
    WSC = 256.0            # fp8 scale for the attention-weights row

    nc = bacc.Bacc("TRN2", target_bir_lowering=False, debug=debug)

    x = nc.dram_tensor("inputs", [bpc, s, h], F32, kind="ExternalInput").ap()
    W_p = nc.dram_tensor("W_p", [h, h], F32, kind="ExternalInput").ap()
    v_p = nc.dram_tensor("v_p", [h, 1], F32, kind="ExternalInput").ap()
    W_a = nc.dram_tensor("W_a", [h, h], F32, kind="ExternalInput").ap()
    U_a = nc.dram_tensor("U_a", [h, h], F32, kind="ExternalInput").ap()
    v_a = nc.dram_tensor("v_a", [h, 1], F32, kind="ExternalInput").ap()
    W_att = nc.dram_tensor("W_att", [2 * h, size], F32, kind="ExternalInput").ap()
    out = nc.dram_tensor("out", [bpc, size], F32, kind="ExternalOutput").ap()

    with tile.TileContext(nc) as tc, ExitStack() as ctx:
        sb = ctx.enter_context(tc.tile_pool(name="sb", bufs=1))
        ps = ctx.enter_context(tc.tile_pool(name="ps", bufs=1, space="PSUM"))

        dp = ctx.enter_context(tc.tile_pool(name="dram", bufs=3,
                                            space="DRAM"))
        NJ = h // 256          # 4 DoubleRow k-blocks of 256
        NCH = 8                # x chunks per batch (256 s-rows each)
        CS = s // NCH
        xt_tiles = [None] * bpc
        xf8 = [None] * bpc

        # ---- x pipeline, phase-major: loads -> DVE casts -> stores ----
        x32_tiles = {}
        x8c_tiles = {}

        def emit_x_loads(b, split=False):
            xf8[b] = [dp.tile([s // 2, h], F8, name=f"xf8_{b}_{hf}",
                              tag=f"xf8{hf}") for hf in range(2)]
            for c in range(NCH):
                x32c = sb.tile([P, CS // P, h], F32, name=f"x32_{b}_{c}",
                               tag="x32", bufs=3)
                eng = nc.scalar if (split and c % 2 == 1) else nc.gpsimd
                eng.dma_start(
                    x32c[:], x[b, c * CS:(c + 1) * CS, :]
                    .rearrange("(p si) h -> p si h", p=P))
                x32_tiles[(b, c)] = x32c

        def emit_x_cast_store(b, c):
            x8c = sb.tile([P, CS // P, h], F8, name=f"x8_{b}_{c}",
                          tag="x8c", bufs=2)
            nc.vector.tensor_copy(x8c[:], x32_tiles.pop((b, c))[:])
            hf, cc = divmod(c, NCH // 2)
            nc.sync.dma_start(
                xf8[b][hf][cc * CS:(cc + 1) * CS, :]
                .rearrange("(p si) h -> p si h", p=P), x8c[:])

        def emit_x_casts(b):
            for c in range(NCH):
                emit_x_cast_store(b, c)

        def emit_x_stores(b):
            pass

        # u16-view transposes of the fp8 DRAM image (2 s-halves x 4 j)
        def emit_transposes_half(b, c2):
            if xt_tiles[b] is None or c2 == 0:
                xt_tiles[b] = sb.tile([P, NJ, s], U16, name=f"xt_{b}",
                                      tag="xt", bufs=2)
            xt = xt_tiles[b]
            xu = xf8[b][c2][:].bitcast(U16)  # [s//2, h//2] u16
            for j in range(NJ):
                nc.sync.dma_start(
                    xt[:, j, c2 * (s // 2):(c2 + 1) * (s // 2)],
                    xu[:, j * P:(j + 1) * P],
                    transpose=True)

        def emit_transposes(b):
            emit_transposes_half(b, 0)
            emit_transposes_half(b, 1)

        def emit_xt(b):
            emit_x_loads(b)
            emit_x_casts(b)
            emit_x_stores(b)

        # ---- weight staging: plain fp32 loads + engine casts ----
        # scalar HWDGE queue: U_a, W_p halves; gpsimd SWDGE queue
        # (plain, no cast): W_a, W_att halves.
        wa8 = sb.tile([P, NJ, 2, h], F8, name="wa8", tag="wa8")
        for hh in range(NJ):
            st = sb.tile([P, 2, h], F32, name=f"wa32_{hh}",
                         tag="wst", bufs=2)
            nc.gpsimd.dma_start(
                st[:], W_a.rearrange("(j p two) m -> p j two m",
                                     j=NJ, p=P)[:, hh, :, :])
            nc.vector.tensor_copy(wa8[:, hh, :, :], st[:])

        # ---- no-dependency constants first (keeps DVE FIFO clear) ----
        chan4 = sb.tile([16, P], F32, name="chan4", tag="chan4")
        nc.gpsimd.iota(chan4[:], pattern=[[0, P]], base=0,
                       channel_multiplier=1,
                       allow_small_or_imprecise_dtypes=True)
        ident = sb.tile([bpc, bpc], F32, name="ident", tag="ident")
        nc.gpsimd.iota(ident[:], pattern=[[1, bpc]], base=0,
                       channel_multiplier=-1,
                       allow_small_or_imprecise_dtypes=True)
        pos4 = sb.tile([SQ, NB], F16, name="pos4", tag="pos")
        nc.gpsimd.iota(pos4[:], pattern=[[1, NB]], base=0,
                       channel_multiplier=NB,
                       allow_small_or_imprecise_dtypes=True)
        nc.vector.tensor_scalar(ident[:], ident[:], 0.0, None,
                                op0=ALU.is_equal)
        ones1 = sb.tile([bpc, P], F16, name="ones1", tag="ones1")
        nc.vector.memset(ones1[:], 1.0)
        onesc = sb.tile([1, bpc], F16, name="onesc", tag="onesc")
        nc.vector.memset(onesc[:], 1.0)
        sels = []
        for q in range(SQ):
            sq = sb.tile([SQ, P], F16, name=f"sel{q}", tag="sel", bufs=SQ)
            nc.vector.tensor_scalar(sq[:], chan4[0:SQ, :], float(q), None,
                                    op0=ALU.is_equal)
            sels.append(sq)

        # ---- x[0] pipeline + h_t row (htb kicked before the odd loads) ----
        htb = sb.tile([bpc, h], F32, name="htb", tag="htb")
        nc.scalar.dma_start(htb[:], x[:, s - 1, :])
        emit_x_loads(0, split=True)
        for c in range(NCH // 2):
            emit_x_cast_store(0, c)
        emit_transposes_half(0, 0)
        for c in range(NCH // 2, NCH):
            emit_x_cast_store(0, c)
        emit_transposes_half(0, 1)

        # ---- h_t transposes -> htT16 [P, KT, bpc], combT h_t half ----
        htT16 = sb.tile([P, KT, bpc], F16, name="htT16", tag="htT16")
        combT = sb.tile([P, KT2, bpc], F16, name="combT", tag="combT")
        for k in range(KT):
            pt = ps.tile([P, bpc], F32, name=f"pt_{k}", tag="misc", bufs=1)
            nc.tensor.transpose(pt[:], htb[:, k * P:(k + 1) * P], ident[:])
            nc.vector.tensor_copy(htT16[:, k, :], pt[:])
            nc.vector.tensor_copy(combT[:, KT + k, :], pt[:])

        # ---- wt = h_t @ U_a -> wtT, column-block streamed so the first
        # tanh only needs the first 1MB of U_a ----
        NWC = 4                   # column chunks of 256
        WCW = h // NWC
        wt_row = sb.tile([bpc, h], F32, name="wt_row", tag="wt_row")
        wtT = sb.tile([P, KT, bpc], F32, name="wtT", tag="wtT")

        def emit_wcol_chunk(wap, nn, dest_row, dest_T, act_fn):
            st = sb.tile([P, KT, WCW], F32, name=f"wc32_{id(wap)}_{nn}",
                         tag="wst2", bufs=2)
            nc.gpsimd.dma_start(
                st[:], wap.rearrange("(k p) n -> p k n", p=P)
                [:, :, nn * WCW:(nn + 1) * WCW])
            c16 = sb.tile([P, KT, WCW], F16, name=f"wc16_{id(wap)}_{nn}",
                          tag="w16c", bufs=2)
            nc.vector.tensor_copy(c16[:], st[:])
            pw = ps.tile([bpc, WCW], F32, name=f"pw_{id(wap)}_{nn}",
                         tag="pk", bufs=2)
            for k in range(KT):
                nc.tensor.matmul(pw[:], htT16[:, k, :], c16[:, k, :],
                                 start=(k == 0), stop=(k == KT - 1),
                                 skip_group_check=True)
            nc.scalar.activation(dest_row[:, nn * WCW:(nn + 1) * WCW],
                                 pw[:], act_fn)
            if dest_T is not None:
                for z in range(WCW // P):
                    kk = nn * (WCW // P) + z
                    pt2 = ps.tile([P, bpc], F32, name=f"ptT_{nn}_{z}",
                                  tag="misc", bufs=1)
                    nc.tensor.transpose(
                        pt2[:], dest_row[:, kk * P:(kk + 1) * P], ident[:])
                    nc.scalar.activation(dest_T[:, kk, :], pt2[:], AF.Copy)

        for nn in range(NWC):
            emit_wcol_chunk(U_a, nn, wt_row, wtT, AF.Copy)

        # v_a / v_p loads + vaq build
        vp_rep = sb.tile([bpc, h], F32, name="vp_rep", tag="vp_rep")
        for i in range(bpc):
            nc.scalar.dma_start(vp_rep[i:i + 1, :], v_p.rearrange("n o -> o n"))
        va32 = sb.tile([P, KT, 1], F32, name="va32", tag="va32")
        nc.scalar.dma_start(va32[:], v_a.rearrange("(a p) o -> p a o", p=P))
        vaqs = []
        for q in range(SQ):
            vq = sb.tile([P, KT, 16], F8, name=f"vaq{q}", tag="vaq", bufs=4)
            nc.vector.memset(vq[:], 0.0)
            nc.vector.tensor_copy(vq[:, :, q:q + 1], va32[:])
            vaqs.append(vq)

        emit_xt(1)
        emit_transposes(1)

        # ---- p_t = sigmoid(tanh(h_t @ W_p) @ v_p) * s -> pbT ----
        tanhP = sb.tile([bpc, h], F32, name="tanhP", tag="tanhP")
        for nn in range(NWC):
            emit_wcol_chunk(W_p, nn, tanhP, None, AF.Tanh)
        z2t = sb.tile([bpc, 1], F32, name="z2t", tag="z2t")
        nc.vector.scalar_tensor_tensor(
            tanhP[:], tanhP[:], 1.0, vp_rep[:],
            op0=ALU.mult, op1=ALU.mult, accum_out=z2t[:])
        pz = ps.tile([1, bpc], F32, name="pz", tag="misc", bufs=1)
        nc.tensor.transpose(pz[:], z2t[:], ident[:])
        p_row = sb.tile([1, bpc], F16, name="p_row", tag="p_row")
        nc.scalar.activation(p_row[:], pz[:], AF.Sigmoid)
        pbt_ps = ps.tile([bpc, bpc], F32, name="pbt_ps", tag="misc", bufs=1)
        nc.tensor.matmul(pbt_ps[:], onesc[:], p_row[:], start=True, stop=True)
        pbT = sb.tile([bpc, bpc], F32, name="pbT", tag="pbT")
        nc.scalar.activation(pbT[:], pbt_ps[:], AF.Copy, scale=float(s))

        # ---- W_att fp16 (emitted during the batch loop) ----
        watt_holder = [None]

        def emit_watt_half(part):
            if part == "ht":
                w16 = sb.tile([P, KT2, size], F16, name="watt16", tag="watt")
                watt_holder[0] = w16
                src_ap = W_att[h:2 * h, :].rearrange("(k p) n -> p k n", p=P)
            else:
                w16 = watt_holder[0]
                src_ap = W_att[0:h, :].rearrange(
                    "(j p two) n -> p j two n", j=NJ, p=P)
            base = KT if part == "ht" else 0
            for hh in range(NJ):
                st = sb.tile([P, 2, size], F32, name=f"wat32{part}_{hh}",
                             tag="wst", bufs=2)
                if part == "ht":
                    nc.gpsimd.dma_start(st[:],
                                        src_ap[:, 2 * hh:2 * hh + 2, :])
                else:
                    nc.gpsimd.dma_start(st[:], src_ap[:, hh, :, :])
                if hh % 2 == 0:
                    nc.vector.tensor_copy(
                        w16[:, base + 2 * hh:base + 2 * hh + 2, :], st[:])
                else:
                    nc.scalar.activation(
                        w16[:, base + 2 * hh:base + 2 * hh + 2, :].rearrange(
                            "p a n -> p (a n)"),
                        st[:].rearrange("p a n -> p (a n)"),
                        mybir.ActivationFunctionType.Copy)

        def emit_watt_ht():
            emit_watt_half("ht")

        def emit_watt_ctx():
            emit_watt_half("ctx")

        # ---- main batch loop ----
        htpart = sb.tile([bpc, size], F32, name="htpart", tag="htpart")

        def emit_final_ht():
            w16 = watt_holder[0]
            for n2 in range(NO):
                pf = ps.tile([bpc, NB], F32, name=f"pfh_{n2}", tag="pk",
                             bufs=2)
                for kk in range(KT, KT2):
                    nc.tensor.matmul(pf[:], combT[:, kk, :],
                                     w16[:, kk, n2 * NB:(n2 + 1) * NB],
                                     start=(kk == KT), stop=(kk == KT2 - 1),
                                     skip_group_check=True)
                nc.scalar.activation(htpart[:, n2 * NB:(n2 + 1) * NB],
                                     pf[:], AF.Copy)

        sc_tiles = {}
        xt_of = {}

        def emit_batch_tail(b):
            sc_ps = sc_tiles.pop(b)
            xt = xt_of.pop(b)
            score4 = sc_ps[0:SQ, :]
            e4 = sb.tile([SQ, NB], F16, name=f"e4_{b}", tag="e4", bufs=2)
            nc.scalar.activation(e4[:], score4, AF.Exp)
            zp = sb.tile([SQ, 1], F16, name=f"zp_{b}", tag="zp", bufs=2)
            with nc.allow_low_precision(reason="Z fits fp16 comfortably"):
                nc.vector.tensor_reduce(zp[:], e4[:], axis=AX.X, op=ALU.add)
            zs_ps = ps.tile([1, 1], F32, name=f"zs_{b}", tag="misc", bufs=1)
            nc.tensor.matmul(zs_ps[:], zp[:], ones1[0:SQ, 0:1],
                             start=True, stop=True, skip_group_check=True)
            rr = sb.tile([1, 1], F16, name=f"rr_{b}", tag="rr", bufs=2)
            with nc.allow_low_precision(reason="1/Z fits fp16"):
                nc.vector.reciprocal(rr[:], zs_ps[:])
            rr_ps = ps.tile([SQ, 1], F32, name=f"rrp_{b}", tag="misc", bufs=1)
            nc.tensor.matmul(rr_ps[:], onesc[0:1, 0:SQ], rr[:],
                             start=True, stop=True, skip_group_check=True)
            rr4 = sb.tile([SQ, 1], F32, name=f"rr4_{b}", tag="rr4", bufs=2)
            nc.scalar.activation(rr4[:], rr_ps[:], AF.Copy, scale=WSC)
            t4 = sb.tile([SQ, NB], F16, name=f"t4_{b}", tag="t4", bufs=2)
            nc.vector.tensor_scalar(t4[:], pos4[:], pbT[0:SQ, b:b + 1],
                                    inv_sq_denom, op0=ALU.subtract,
                                    op1=ALU.mult)
            d2n = sb.tile([SQ, NB], F16, name=f"d2_{b}", tag="d2", bufs=2)
            nc.vector.scalar_tensor_tensor(d2n[:], t4[:], -1.0, t4[:],
                                           op0=ALU.mult, op1=ALU.mult)
            gr = sb.tile([SQ, NB], F16, name=f"gr_{b}", tag="gr", bufs=2)
            nc.scalar.activation(gr[:], d2n[:], AF.Exp)
            wu4 = sb.tile([SQ, NB], F8, name=f"wu_{b}", tag="wu", bufs=2)
            nc.vector.scalar_tensor_tensor(wu4[:], e4[:], rr4[:], gr[:],
                                           op0=ALU.mult, op1=ALU.mult)

            wrep = sb.tile([P, s], F8, name=f"wrep_{b}", tag="wrep", bufs=2)
            for q in range(SQ):
                pwr = ps.tile([P, NB], F32, name=f"pwr_{b}_{q}", tag="pk",
                              bufs=2)
                nc.tensor.matmul(pwr[:], sels[q][:], wu4[:],
                                 start=True, stop=True, skip_group_check=True)
                nc.scalar.activation(wrep[:, q * NB:(q + 1) * NB], pwr[:],
                                     AF.Copy)

            junk = sb.tile([P, s], F8, name=f"junk_{b}", tag="junk", bufs=1)
            ctxa = sb.tile([P, NJ, 2], F32, name=f"ctxa_{b}", tag="ctxa",
                           bufs=2)
            for j in range(NJ):
                xv8 = xt[:, j, :].bitcast(F8).rearrange(
                    "p (n two) -> p n two", two=2)
                for i in range(2):
                    nc.vector.scalar_tensor_tensor(
                        junk[:], xv8[:, :, i], 1.0, wrep[:],
                        op0=ALU.mult, op1=ALU.mult,
                        accum_out=ctxa[:, j, i:i + 1])
            nc.vector.tensor_scalar_mul(
                combT[:, 0:KT, b:b + 1].rearrange("p k o -> p (k o)"),
                ctxa[:].rearrange("p j i -> p (j i)"), 1.0 / WSC)

        for b in range(bpc):
            if b + 2 < bpc:
                emit_x_loads(b + 2)
            xt = xt_tiles[b]
            rhs_j = [(xt[:, j, :].bitcast(F8)
                      .rearrange("p (n two) -> p n two", two=2)
                      .rearrange("p n two -> p two n"))
                     for j in range(NJ)]

            th8 = sb.tile([P, KT, s], F8, name=f"th_{b}", tag="tanh", bufs=1)
            sc_ps = ps.tile([16, NB], F32, name=f"sc_{b}", tag="sc", bufs=1)
            sc_tiles[b] = sc_ps
            xt_of[b] = xt

            def emit_va_mms(a):
                for q in range(SQ):
                    nc.tensor.matmul(
                        sc_ps[:], vaqs[q][:, 2 * a:2 * a + 2, :],
                        th8[:, 2 * a:2 * a + 2, q * NB:(q + 1) * NB],
                        start=(a == 0 and q == 0),
                        stop=(a == KT // 2 - 1 and q == SQ - 1),
                        perf_mode=DR, skip_group_check=True)

            for hp in range(KT):
                for sh in range(2):
                    wh = ps.tile([P, 2 * NB], F32, name=f"wh_{b}_{hp}_{sh}",
                                 tag="wh", bufs=2)
                    for j in range(NJ):
                        lhsT = wa8[:, j, :, hp * P:(hp + 1) * P]
                        for q2 in range(2):
                            s0 = sh * 2 * NB + q2 * NB
                            nc.tensor.matmul(
                                wh[:, q2 * NB:(q2 + 1) * NB], lhsT,
                                rhs_j[j][:, :, s0:s0 + NB],
                                start=(j == 0), stop=(j == NJ - 1),
                                perf_mode=DR, skip_group_check=True)
                    nc.scalar.activation(
                        th8[:, hp, sh * 2 * NB:(sh + 1) * 2 * NB], wh[:],
                        AF.Tanh, bias=wtT[:, hp, b:b + 1])
                if b + 2 < bpc and hp < 4:
                    emit_x_cast_store(b + 2, 2 * hp)
                    emit_x_cast_store(b + 2, 2 * hp + 1)
                if hp == 2 and b >= 1:
                    emit_batch_tail(b - 1)
                if hp == 4 and b + 2 < bpc:
                    emit_transposes_half(b + 2, 0)
                if hp % 2 == 1 and hp >= 3:
                    emit_va_mms(hp // 2 - 1)
            emit_va_mms(KT // 2 - 1)
            if b + 2 < bpc:
                emit_transposes_half(b + 2, 1)
            if b == 0:
                emit_watt_ht()
            if b == 1:
                emit_watt_ctx()
                emit_final_ht()
            if b == bpc - 1:
                emit_batch_tail(b)

        # ---- final projection tail: ctx half + h_t part, then tanh ----
        watt16 = watt_holder[0]
        outsb = sb.tile([bpc, size], F32, name="outsb", tag="outsb")
        pfs = [ps.tile([bpc, NB], F32, name=f"pf_{n2}", tag="pk", bufs=2)
               for n2 in range(NO)]
        for kk in range(KT):
            for n2 in range(NO):
                nc.tensor.matmul(pfs[n2][:], combT[:, kk, :],
                                 watt16[:, kk, n2 * NB:(n2 + 1) * NB],
                                 start=(kk == 0), stop=(kk == KT - 1),
                                 skip_group_check=True)
        for n2 in range(NO):
            nc.vector.tensor_tensor(
                outsb[:, n2 * NB:(n2 + 1) * NB], pfs[n2][:],
                htpart[:, n2 * NB:(n2 + 1) * NB], op=ALU.add)
            nc.scalar.activation(outsb[:, n2 * NB:(n2 + 1) * NB],
                                 outsb[:, n2 * NB:(n2 + 1) * NB], AF.Tanh)
        nc.scalar.dma_start(out[:], outsb[:])

    nc.compile()
    return nc


_ldw_patched = False


def _enable_ldw_opt():
    # walrus dedups back-to-back LDWEIGHTS of the same stationary tile;
    # concourse pins it off. Each DoubleRow matmul otherwise pays a
    # 107ns weight reload that the reorder window cannot hide.
    global _ldw_patched
    if _ldw_patched:
        return
    from concourse import bass_utils

    orig = bass_utils.run_command

    def patched(cmd, *a, **kw):
        cmd = ["--enable-ldw-opt=true" if c == "--enable-ldw-opt=false"
               else c for c in cmd]
        return orig(cmd, *a, **kw)

    bass_utils.run_command = patched
    _ldw_patched = True


def kernel(**inputs):
    global _compiled
    from concourse import bass_utils

    if _compiled is None:
        _compiled = _build()

    x = np.ascontiguousarray(np.asarray(inputs["inputs"], dtype=np.float32))
    weights = {
        k: np.ascontiguousarray(np.asarray(inputs[k], dtype=np.float32))
        for k in ("W_p", "v_p", "W_a", "U_a", "v_a", "W_att")
    }
    in_maps = [
        {"inputs": x[i * BPC:(i + 1) * BPC], **weights} for i in range(N_CORES)
    ]
    res = bass_utils.run_bass_kernel_spmd(_compiled, in_maps,
                                          list(range(N_CORES)))
    return np.concatenate([res.results[i]["out"] for i in range(N_CORES)],
                          axis=0).astype(np.float32)


# revision 39
# speedup vs baseline: 1.1513x; 1.1513x over previous
"""Trainium2 Bass kernel: Bahdanau local-p attention (B=32, S=2048, H=1024).

Sharding: data-parallel over batch. Each of the 8 cores processes B/8 = 4
batches end-to-end (weights replicated); no collectives.

Key insight vs the naive pipeline: SWDGE cast-DMAs are element-rate
limited (~20 G elem/s -> ~90us per 2.1M-element cast), so no
dtype-converting DMA is ever issued. Per-core dataflow:
  1. x[b] fp32 loaded natural in [256-row] chunks (plain DMA, full
     rate), cast fp32->fp8e4 on DVE/GpSimd, stored fp8 to DRAM (plain),
     then u16-viewed xbar transposes -> SBUF xt [128, 4j, 2048] u16,
     pair-interleaved: partition p of block j holds bytes
     (h=256j+2p, h=256j+2p+1) along s.
  2. Main matmul in fp8 DoubleRow (2x PE rate): lhsT = W_a [128,2,128]
     in matching (j p two) order, rhs = interleaved fp8 view
     [128, 2(stride 1B), 512(stride 2B)]; K=256 per instruction.
  3. tanh(WH^T + U_a h_t) on ACT with per-partition bias, fp8 out.
  4. score via DoubleRow v_a matmuls; s-block q accumulates into row q
     of one [16,512] PSUM bank (v_a placed in weight column q).
  5. softmax (no max-sub; scores are O(1)) * gaussian on [4,512] rows;
     weight row scaled by 256 for fp8 range.
  6. selector-matmul replicates the weight row to 128 partitions;
     context via 8 strided fp8 DVE multiply-accumulates (one per (j,i)).
  7. final tanh([ctx, h_t] @ W_att) in fp16; W_att ctx rows loaded in
     the same (j p two) permuted order so the context needs no shuffle.
All weights are plain fp32 DMA loads + engine casts (fp16/fp8).
"""

import math
from contextlib import ExitStack

import numpy as np

B, S, H, SIZE = 32, 2048, 1024, 1024
N_CORES = 8
BPC = B // N_CORES
P = 128
NB = 512

_compiled = None


def _build(bpc=BPC, s=S, h=H, size=SIZE, debug=False):
    import concourse.bacc as bacc
    import concourse.mybir as mybir
    import concourse.tile as tile

    F32 = mybir.dt.float32
    F16 = mybir.dt.float16
    BF16 = mybir.dt.bfloat16
    F8 = mybir.dt.float8e4
    U16 = mybir.dt.uint16
    AF = mybir.ActivationFunctionType
    ALU = mybir.AluOpType
    AX = mybir.AxisListType
    DR = mybir.MatmulPerfMode.DoubleRow

    KT = h // P            # 8 h-tiles of 128
    SQ = s // NB           # 4 s-blocks of 512
    KT2 = 2 * h // P       # 16 k-tiles for the final projection
    NO = size // NB        # 2 output blocks of 512
    denom = 2.0 * ((s // 2) / 2.0) ** 2
    inv_sq_denom = 1.0 / math.sqrt(denom)
    WSC = 256.0            # fp8 scale for the attention-weights row

    nc = bacc.Bacc("TRN2", target_bir_lowering=False, debug=debug)

    x = nc.dram_tensor("inputs", [bpc, s, h], F32, kind="ExternalInput").ap()
    W_p = nc.dram_tensor("W_p", [h, h], F32, kind="ExternalInput").ap()
    v_p = nc.dram_tensor("v_p", [h, 1], F32, kind="ExternalInput").ap()
    W_a = nc.dram_tensor("W_a", [h, h], F32, kind="ExternalInput").ap()
    U_a = nc.dram_tensor("U_a", [h, h], F32, kind="ExternalInput").ap()
    v_a = nc.dram_tensor("v_a", [h, 1], F32, kind="ExternalInput").ap()
    W_att = nc.dram_tensor("W_att", [2 * h, size], F32, kind="ExternalInput").ap()
    out = nc.dram_tensor("out", [bpc, size], F32, kind="ExternalOutput").ap()

    with tile.TileContext(nc) as tc, ExitStack() as ctx:
        sb = ctx.enter_context(tc.tile_pool(name="sb", bufs=1))
        ps = ctx.enter_context(tc.tile_pool(name="ps", bufs=1, space="PSUM"))

        dp = ctx.enter_context(tc.tile_pool(name="dram", bufs=3,
                                            space="DRAM"))
        NJ = h // 256          # 4 DoubleRow k-blocks of 256
        NCH = 8                # x chunks per batch (256 s-rows each)
        CS = s // NCH
        xt_tiles = [None] * bpc
        xf8 = [None] * bpc

        # ---- x pipeline, phase-major: loads -> DVE casts -> stores ----
        x32_tiles = {}
        x8c_tiles = {}

        def emit_x_loads(b, split=False):
            xf8[b] = [dp.tile([s // 2, h], F8, name=f"xf8_{b}_{hf}",
                              tag=f"xf8{hf}") for hf in range(2)]
            for c in range(NCH):
                x32c = sb.tile([P, CS // P, h], F32, name=f"x32_{b}_{c}",
                               tag="x32", bufs=3)
                eng = nc.scalar if (split and c % 2 == 1) else nc.gpsimd
                eng.dma_start(
                    x32c[:], x[b, c * CS:(c + 1) * CS, :]
                    .rearrange("(p si) h -> p si h", p=P))
                x32_tiles[(b, c)] = x32c

        def emit_x_cast_store(b, c):
            x8c = sb.tile([P, CS // P, h], F8, name=f"x8_{b}_{c}",
                          tag="x8c", bufs=2)
            nc.vector.tensor_copy(x8c[:], x32_tiles.pop((b, c))[:])
            hf, cc = divmod(c, NCH // 2)
            nc.sync.dma_start(
                xf8[b][hf][cc * CS:(cc + 1) * CS, :]
                .rearrange("(p si) h -> p si h", p=P), x8c[:])

        def emit_x_casts(b):
            for c in range(NCH):
                emit_x_cast_store(b, c)

        def emit_x_stores(b):
            pass

        # u16-view transposes of the fp8 DRAM image (2 s-halves x 4 j)
        def emit_transposes_half(b, c2):
            if xt_tiles[b] is None or c2 == 0:
                xt_tiles[b] = sb.tile([P, NJ, s], U16, name=f"xt_{b}",
                                      tag="xt", bufs=2)
            xt = xt_tiles[b]
            xu = xf8[b][c2][:].bitcast(U16)  # [s//2, h//2] u16
            for j in range(NJ):
                nc.sync.dma_start(
                    xt[:, j, c2 * (s // 2):(c2 + 1) * (s // 2)],
                    xu[:, j * P:(j + 1) * P],
                    transpose=True)

        def emit_transposes(b):
            emit_transposes_half(b, 0)
            emit_transposes_half(b, 1)

        def emit_xt(b):
            emit_x_loads(b)
            emit_x_casts(b)
            emit_x_stores(b)

        # ---- weight staging: plain fp32 loads + engine casts ----
        # scalar HWDGE queue: U_a, W_p halves; gpsimd SWDGE queue
        # (plain, no cast): W_a, W_att halves.
        wa8 = sb.tile([P, NJ, 2, h], F8, name="wa8", tag="wa8")
        for hh in range(NJ):
            st = sb.tile([P, 2, h], F32, name=f"wa32_{hh}",
                         tag="wst", bufs=2)
            nc.gpsimd.dma_start(
                st[:], W_a.rearrange("(j p two) m -> p j two m",
                                     j=NJ, p=P)[:, hh, :, :])
            nc.vector.tensor_copy(wa8[:, hh, :, :], st[:])

        # ---- no-dependency constants first (keeps DVE FIFO clear) ----
        chan4 = sb.tile([16, P], F32, name="chan4", tag="chan4")
        nc.gpsimd.iota(chan4[:], pattern=[[0, P]], base=0,
                       channel_multiplier=1,
                       allow_small_or_imprecise_dtypes=True)
        ident = sb.tile([bpc, bpc], F32, name="ident", tag="ident")
        nc.gpsimd.iota(ident[:], pattern=[[1, bpc]], base=0,
                       channel_multiplier=-1,
                       allow_small_or_imprecise_dtypes=True)
        pos4 = sb.tile([SQ, NB], F16, name="pos4", tag="pos")
        nc.gpsimd.iota(pos4[:], pattern=[[1, NB]], base=0,
                       channel_multiplier=NB,
                       allow_small_or_imprecise_dtypes=True)
        nc.vector.tensor_scalar(ident[:], ident[:], 0.0, None,
                                op0=ALU.is_equal)
        ones1 = sb.tile([bpc, P], F16, name="ones1", tag="ones1")
        nc.vector.memset(ones1[:], 1.0)
        onesc = sb.tile([1, bpc], F16, name="onesc", tag="onesc")
        nc.vector.memset(onesc[:], 1.0)
        sels = []
        for q in range(SQ):
            sq = sb.tile([SQ, P], F16, name=f"sel{q}", tag="sel", bufs=SQ)
            nc.vector.tensor_scalar(sq[:], chan4[0:SQ, :], float(q), None,
                                    op0=ALU.is_equal)
            sels.append(sq)

        # ---- x[0] pipeline + h_t row (htb kicked before the odd loads) ----
        htb = sb.tile([bpc, h], F32, name="htb", tag="htb")
        nc.scalar.dma_start(htb[:], x[:, s - 1, :])
        emit_x_loads(0, split=True)
        for c in range(NCH // 2):
            emit_x_cast_store(0, c)
        emit_transposes_half(0, 0)
        for c in range(NCH // 2, NCH):
            emit_x_cast_store(0, c)
        emit_transposes_half(0, 1)

        # ---- h_t transposes -> htT16 [P, KT, bpc], combT h_t half ----
        htT16 = sb.tile([P, KT, bpc], F16, name="htT16", tag="htT16")
        combT = sb.tile([P, KT2, bpc], F16, name="combT", tag="combT")
        for k in range(KT):
            pt = ps.tile([P, bpc], F32, name=f"pt_{k}", tag="misc", bufs=1)
            nc.tensor.transpose(pt[:], htb[:, k * P:(k + 1) * P], ident[:])
            nc.vector.tensor_copy(htT16[:, k, :], pt[:])
            nc.vector.tensor_copy(combT[:, KT + k, :], pt[:])

        # ---- wt = h_t @ U_a -> wtT, column-block streamed so the first
        # tanh only needs the first 1MB of U_a ----
        NWC = 4                   # column chunks of 256
        WCW = h // NWC
        wt_row = sb.tile([bpc, h], F32, name="wt_row", tag="wt_row")
        wtT = sb.tile([P, KT, bpc], F32, name="wtT", tag="wtT")

        def emit_wcol_chunk(wap, nn, dest_row, dest_T, act_fn):
            st = sb.tile([P, KT, WCW], F32, name=f"wc32_{id(wap)}_{nn}",
                         tag="wst2", bufs=2)
            nc.gpsimd.dma_start(
                st[:], wap.rearrange("(k p) n -> p k n", p=P)
                [:, :, nn * WCW:(nn + 1) * WCW])
            c16 = sb.tile([P, KT, WCW], F16, name=f"wc16_{id(wap)}_{nn}",
                          tag="w16c", bufs=2)
            nc.vector.tensor_copy(c16[:], st[:])
            pw = ps.tile([bpc, WCW], F32, name=f"pw_{id(wap)}_{nn}",
                         tag="pk", bufs=2)
            for k in range(KT):
                nc.tensor.matmul(pw[:], htT16[:, k, :], c16[:, k, :],
                                 start=(k == 0), stop=(k == KT - 1),
                                 skip_group_check=True)
            nc.scalar.activation(dest_row[:, nn * WCW:(nn + 1) * WCW],
                                 pw[:], act_fn)
            if dest_T is not None:
                for z in range(WCW // P):
                    kk = nn * (WCW // P) + z
                    pt2 = ps.tile([P, bpc], F32, name=f"ptT_{nn}_{z}",
                                  tag="misc", bufs=1)
                    nc.tensor.transpose(
                        pt2[:], dest_row[:, kk * P:(kk + 1) * P], ident[:])
                    nc.scalar.activation(dest_T[:, kk, :], pt2[:], AF.Copy)

        for nn in range(NWC):
            emit_wcol_chunk(U_a, nn, wt_row, wtT, AF.Copy)

        # v_a / v_p loads + vaq build
        vp_rep = sb.tile([bpc, h], F32, name="vp_rep", tag="vp_rep")
        for i in range(bpc):
            nc.scalar.dma_start(vp_rep[i:i + 1, :], v_p.rearrange("n o -> o n"))
        va32 = sb.tile([P, KT, 1], F32, name="va32", tag="va32")
        nc.scalar.dma_start(va32[:], v_a.rearrange("(a p) o -> p a o", p=P))
        vaqs = []
        for q in range(SQ):
            vq = sb.tile([P, KT, 16], F8, name=f"vaq{q}", tag="vaq", bufs=4)
            nc.vector.memset(vq[:], 0.0)
            nc.vector.tensor_copy(vq[:, :, q:q + 1], va32[:])
            vaqs.append(vq)

        emit_xt(1)
        emit_transposes(1)

        # ---- p_t = sigmoid(tanh(h_t @ W_p) @ v_p) * s -> pbT ----
        tanhP = sb.tile([bpc, h], F32, name="tanhP", tag="tanhP")
        for nn in range(NWC):
            emit_wcol_chunk(W_p, nn, tanhP, None, AF.Tanh)
        z2t = sb.tile([bpc, 1], F32, name="z2t", tag="z2t")
        nc.vector.scalar_tensor_tensor(
            tanhP[:], tanhP[:], 1.0, vp_rep[:],
            op0=ALU.mult, op1=ALU.mult, accum_out=z2t[:])
        pz = ps.tile([1, bpc], F32, name="pz", tag="misc", bufs=1)
        nc.tensor.transpose(pz[:], z2t[:], ident[:])
        p_row = sb.tile([1, bpc], F16, name="p_row", tag="p_row")
        nc.scalar.activation(p_row[:], pz[:], AF.Sigmoid)
        pbt_ps = ps.tile([bpc, bpc], F32, name="pbt_ps", tag="misc", bufs=1)
        nc.tensor.matmul(pbt_ps[:], onesc[:], p_row[:], start=True, stop=True)
        pbT = sb.tile([bpc, bpc], F32, name="pbT", tag="pbT")
        nc.scalar.activation(pbT[:], pbt_ps[:], AF.Copy, scale=float(s))

        # ---- W_att fp16 (emitted during the batch loop) ----
        watt_holder = [None]

        def emit_watt_half(part):
            if part == "ht":
                w16 = sb.tile([P, KT2, size], F16, name="watt16", tag="watt")
                watt_holder[0] = w16
                src_ap = W_att[h:2 * h, :].rearrange("(k p) n -> p k n", p=P)
            else:
                w16 = watt_holder[0]
                src_ap = W_att[0:h, :].rearrange(
                    "(j p two) n -> p j two n", j=NJ, p=P)
            base = KT if part == "ht" else 0
            for hh in range(NJ):
                st = sb.tile([P, 2, size], F32, name=f"wat32{part}_{hh}",
                             tag="wst", bufs=2)
                if part == "ht":
                    nc.gpsimd.dma_start(st[:],
                                        src_ap[:, 2 * hh:2 * hh + 2, :])
                else:
                    nc.gpsimd.dma_start(st[:], src_ap[:, hh, :, :])
                if hh % 2 == 0:
                    nc.vector.tensor_copy(
                        w16[:, base + 2 * hh:base + 2 * hh + 2, :], st[:])
                else:
                    nc.scalar.activation(
                        w16[:, base + 2 * hh:base + 2 * hh + 2, :].rearrange(
                            "p a n -> p (a n)"),
                        st[:].rearrange("p a n -> p (a n)"),
                        mybir.ActivationFunctionType.Copy)

        def emit_watt_ht():
            emit_watt_half("ht")

        def emit_watt_ctx():
            emit_watt_half("ctx")

        # ---- main batch loop ----
        htpart = sb.tile([bpc, size], F32, name="htpart", tag="htpart")

        def emit_final_ht():
            w16 = watt_holder[0]
            for n2 in range(NO):
                pf = ps.tile([bpc, NB], F32, name=f"pfh_{n2}", tag="pk",
                             bufs=2)
                for kk in range(KT, KT2):
                    nc.tensor.matmul(pf[:], combT[:, kk, :],
                                     w16[:, kk, n2 * NB:(n2 + 1) * NB],
                                     start=(kk == KT), stop=(kk == KT2 - 1),
                                     skip_group_check=True)
                nc.scalar.activation(htpart[:, n2 * NB:(n2 + 1) * NB],
                                     pf[:], AF.Copy)

        sc_tiles = {}
        xt_of = {}

        def emit_batch_tail(b):
            sc_ps = sc_tiles.pop(b)
            xt = xt_of.pop(b)
            score4 = sc_ps[0:SQ, :]
            e4 = sb.tile([SQ, NB], F16, name=f"e4_{b}", tag="e4", bufs=2)
            nc.scalar.activation(e4[:], score4, AF.Exp)
            zp = sb.tile([SQ, 1], F16, name=f"zp_{b}", tag="zp", bufs=2)
            with nc.allow_low_precision(reason="Z fits fp16 comfortably"):
                nc.vector.tensor_reduce(zp[:], e4[:], axis=AX.X, op=ALU.add)
            zs_ps = ps.tile([1, 1], F32, name=f"zs_{b}", tag="misc", bufs=1)
            nc.tensor.matmul(zs_ps[:], zp[:], ones1[0:SQ, 0:1],
                             start=True, stop=True, skip_group_check=True)
            rr = sb.tile([1, 1], F16, name=f"rr_{b}", tag="rr", bufs=2)
            with nc.allow_low_precision(reason="1/Z fits fp16"):
                nc.vector.reciprocal(rr[:], zs_ps[:])
            rr_ps = ps.tile([SQ, 1], F32, name=f"rrp_{b}", tag="misc", bufs=1)
            nc.tensor.matmul(rr_ps[:], onesc[0:1, 0:SQ], rr[:],
                             start=True, stop=True, skip_group_check=True)
            rr4 = sb.tile([SQ, 1], F32, name=f"rr4_{b}", tag="rr4", bufs=2)
            nc.scalar.activation(rr4[:], rr_ps[:], AF.Copy, scale=WSC)
            t4 = sb.tile([SQ, NB], F16, name=f"t4_{b}", tag="t4", bufs=2)
            nc.vector.tensor_scalar(t4[:], pos4[:], pbT[0:SQ, b:b + 1],
                                    inv_sq_denom, op0=ALU.subtract,
                                    op1=ALU.mult)
            d2n = sb.tile([SQ, NB], F16, name=f"d2_{b}", tag="d2", bufs=2)
            nc.vector.scalar_tensor_tensor(d2n[:], t4[:], -1.0, t4[:],
                                           op0=ALU.mult, op1=ALU.mult)
            gr = sb.tile([SQ, NB], F16, name=f"gr_{b}", tag="gr", bufs=2)
            nc.scalar.activation(gr[:], d2n[:], AF.Exp)
            wu4 = sb.tile([SQ, NB], F8, name=f"wu_{b}", tag="wu", bufs=2)
            nc.vector.scalar_tensor_tensor(wu4[:], e4[:], rr4[:], gr[:],
                                           op0=ALU.mult, op1=ALU.mult)

            wrep = sb.tile([P, s], F8, name=f"wrep_{b}", tag="wrep", bufs=2)
            for q in range(SQ):
                pwr = ps.tile([P, NB], F32, name=f"pwr_{b}_{q}", tag="pk",
                              bufs=2)
                nc.tensor.matmul(pwr[:], sels[q][:], wu4[:],
                                 start=True, stop=True, skip_group_check=True)
                nc.scalar.activation(wrep[:, q * NB:(q + 1) * NB], pwr[:],
                                     AF.Copy)

            junk = sb.tile([P, s], F8, name=f"junk_{b}", tag="junk", bufs=1)
            ctxa = sb.tile([P, NJ, 2], F32, name=f"ctxa_{b}", tag="ctxa",
                           bufs=2)
            for j in range(NJ):
                xv8 = xt[:, j, :].bitcast(F8).rearrange(
                    "p (n two) -> p n two", two=2)
                for i in range(2):
                    nc.vector.scalar_tensor_tensor(
                        junk[:], xv8[:, :, i], 1.0, wrep[:],
                        op0=ALU.mult, op1=ALU.mult,
                        accum_out=ctxa[:, j, i:i + 1])
            nc.vector.tensor_scalar_mul(
                combT[:, 0:KT, b:b + 1].rearrange("p k o -> p (k o)"),
                ctxa[:].rearrange("p j i -> p (j i)"), 1.0 / WSC)

        for b in range(bpc):
            if b + 2 < bpc:
                emit_x_loads(b + 2)
            xt = xt_tiles[b]
            rhs_j = [(xt[:, j, :].bitcast(F8)
                      .rearrange("p (n two) -> p n two", two=2)
                      .rearrange("p n two -> p two n"))
                     for j in range(NJ)]

            th8 = sb.tile([P, KT, s], F8, name=f"th_{b}", tag="tanh", bufs=1)
            sc_ps = ps.tile([16, NB], F32, name=f"sc_{b}", tag="sc", bufs=1)
            sc_tiles[b] = sc_ps
            xt_of[b] = xt

            def emit_va_mms(a):
                for q in range(SQ):
                    nc.tensor.matmul(
                        sc_ps[:], vaqs[q][:, 2 * a:2 * a + 2, :],
                        th8[:, 2 * a:2 * a + 2, q * NB:(q + 1) * NB],
                        start=(a == 0 and q == 0),
                        stop=(a == KT // 2 - 1 and q == SQ - 1),
                        perf_mode=DR, skip_group_check=True)

            for hp in range(KT):
                for sh in range(2):
                    wh = ps.tile([P, 2 * NB], F32, name=f"wh_{b}_{hp}_{sh}",
                                 tag="wh", bufs=2)
                    for j in range(NJ):
                        lhsT = wa8[:, j, :, hp * P:(hp + 1) * P]
                        for q2 in range(2):
                            s0 = sh * 2 * NB + q2 * NB
                            nc.tensor.matmul(
                                wh[:, q2 * NB:(q2 + 1) * NB], lhsT,
                                rhs_j[j][:, :, s0:s0 + NB],
                                start=(j == 0), stop=(j == NJ - 1),
                                perf_mode=DR, skip_group_check=True)
                    nc.scalar.activation(
                        th8[:, hp, sh * 2 * NB:(sh + 1) * 2 * NB], wh[:],
                        AF.Tanh, bias=wtT[:, hp, b:b + 1])
                if b + 2 < bpc and hp < 4:
                    emit_x_cast_store(b + 2, 2 * hp)
                    emit_x_cast_store(b + 2, 2 * hp + 1)
                if hp == 2 and b >= 1:
                    emit_batch_tail(b - 1)
                if hp % 2 == 1 and hp >= 3:
                    emit_va_mms(hp // 2 - 1)
            emit_va_mms(KT // 2 - 1)
            if b + 2 < bpc:
                emit_transposes(b + 2)
            if b == 0:
                emit_watt_ht()
            if b == 1:
                emit_watt_ctx()
                emit_final_ht()
            if b == bpc - 1:
                emit_batch_tail(b)

        # ---- final projection tail: ctx half + h_t part, then tanh ----
        watt16 = watt_holder[0]
        outsb = sb.tile([bpc, size], F32, name="outsb", tag="outsb")
        pfs = [ps.tile([bpc, NB], F32, name=f"pf_{n2}", tag="pk", bufs=2)
               for n2 in range(NO)]
        for kk in range(KT):
            for n2 in range(NO):
                nc.tensor.matmul(pfs[n2][:], combT[:, kk, :],
                                 watt16[:, kk, n2 * NB:(n2 + 1) * NB],
                                 start=(kk == 0), stop=(kk == KT - 1),
                                 skip_group_check=True)
        for n2 in range(NO):
            nc.vector.tensor_tensor(
                outsb[:, n2 * NB:(n2 + 1) * NB], pfs[n2][:],
                htpart[:, n2 * NB:(n2 + 1) * NB], op=ALU.add)
            nc.scalar.activation(outsb[:, n2 * NB:(n2 + 1) * NB],
                                 outsb[:, n2 * NB:(n2 + 1) * NB], AF.Tanh)
        nc.scalar.dma_start(out[:], outsb[:])

    nc.compile()
    return nc


_ldw_patched = False


def _enable_ldw_opt():
    # walrus dedups back-to-back LDWEIGHTS of the same stationary tile;
    # concourse pins it off. Each DoubleRow matmul otherwise pays a
    # 107ns weight reload that the reorder window cannot hide.
    global _ldw_patched
    if _ldw_patched:
        return
    from concourse import bass_utils

    orig = bass_utils.run_command

    def patched(cmd, *a, **kw):
        cmd = ["--enable-ldw-opt=true" if c == "--enable-ldw-opt=false"
               else c for c in cmd]
        return orig(cmd, *a, **kw)

    bass_utils.run_command = patched
    _ldw_patched = True


def kernel(**inputs):
    global _compiled
    from concourse import bass_utils

    if _compiled is None:
        _compiled = _build()

    x = np.ascontiguousarray(np.asarray(inputs["inputs"], dtype=np.float32))
    weights = {
        k: np.ascontiguousarray(np.asarray(inputs[k], dtype=np.float32))
        for k in ("W_p", "v_p", "W_a", "U_a", "v_a", "W_att")
    }
    in_maps = [
        {"inputs": x[i * BPC:(i + 1) * BPC], **weights} for i in range(N_CORES)
    ]
    res = bass_utils.run_bass_kernel_spmd(_compiled, in_maps,
                                          list(range(N_CORES)))
    return np.concatenate([res.results[i]["out"] for i in range(N_CORES)],
                          axis=0).astype(np.float32)
